# revision 15
# baseline (speedup 1.0000x reference)
"""Trainium2 Bass kernel for nn_AvaAttention (GQA attention, head-constant RoPE).

Sharding: tensor-parallel over the 8 kv heads -> core c owns kv head c and
q heads 4c..4c+3. Each core computes its 4 heads' attention and a partial
o_proj (row-split Wo); the host sums the 8 partials.

v2 design (per-core):
- RoPE is head-constant (the module indexes cos/sin by head), so it folds
  into Wq/Wk on the host along with the 1/sqrt(D) scale.
- The whole pipeline runs in bf16 (hidden states DMA'd as bf16 -> half
  the input traffic of the fp32 baseline; q/k/v, exp probs, attn and Wo
  all bf16). Measured end-to-end error ~7e-3, well under the 2e-2 gate;
  PE cost for bf16 is the same 1 col/cycle as fp32r.
  (fp8 DoubleRow + hi/lo residual projections were tried and work
  numerically, but each non-fp32 matmul is split into Ldweights+Matmult,
  and at 107ns/MM the extra ~71ns sequencer dispatch makes the PE
  dispatch-bound - no wall-clock win, so bf16 was kept.)
- Scores are computed transposed ([ktok, qtok]) so exp feeds the PV matmul
  directly; a 65th all-ones column of v makes the PV matmul emit the
  softmax denominator (row 64). exp runs on ScalarE ([128,1024] tiles,
  one per (kb, head-pair)).
- q is stored pair-packed ([64, chunk*2*512]: head-even cols then head-odd
  cols per chunk) so scores need no partition-offset staging.
- Softmax normalization is deferred: denominators are broadcast via a
  tiny PE matmul (emat), reciprocaled on DVE, and multiplied into
  attnT_sb (bf16) which feeds o_proj.
- Projections run up front (8 chunks, PE-bound with fp8 DMA); the 8
  attention units are software-pipelined: unit u's scores/exp overlap
  unit u-1's PV normalize + o_proj.
- PSUM: scores/bcast/po share a 2x[128,1024] ring (4 banks); the two
  per-unit attnT accumulators [65,1024] live in a bufs=2 pool (4 banks);
  projections use their own pools inside a context that exits before the
  attention pools open.
"""

import numpy as np
import ml_dtypes

import concourse.bass as bass
import concourse.bacc as bacc
import concourse.tile as tile
import concourse.mybir as mybir
from concourse import bass_utils

BF16 = mybir.dt.bfloat16
F32 = mybir.dt.float32
F32R = mybir.dt.float32r
FP8 = mybir.dt.float8e4
DR = mybir.MatmulPerfMode.DoubleRow
bf16 = ml_dtypes.bfloat16
e4m3 = ml_dtypes.float8_e4m3

# Problem dims (hardcoded per contract)
B, S, HID = 2, 2048, 2048
NH, KVH, HD = 32, 8, 64
N_CORES = 8


class Dims:
    """All derived tile counts; parameterized so small variants can be
    simulated in CoreSim."""

    def __init__(self, B=B, S=S, HID=HID, n_qheads=4, HD=HD, out_ch=HID):
        self.B, self.S, self.HID, self.HD = B, S, HID, HD
        self.NQ = n_qheads              # q heads per core (must be 4)
        self.BS = B * S                 # total tokens
        self.QCH = n_qheads * HD        # q channels per core (256)
        self.OUT = out_ch               # o_proj output channels
        self.TOK_CHUNK = 512            # projection/attention token chunk
        self.KT = HID // 128            # contraction tiles for projections
        self.NPAIR = self.KT // 2       # DoubleRow k-tile pairs
        self.N_TC = self.BS // self.TOK_CHUNK
        self.N_QC = S // self.TOK_CHUNK  # q chunks per batch
        self.N_KB = S // 128            # ktok blocks per batch
        self.NSB = self.N_KB // 2
        assert n_qheads == 4 and HD == 64
        assert self.KT % 2 == 0 and self.OUT % 1024 == 0 and S % 512 == 0


def build_program(d: Dims, repeat: int = 1, exp_bufs: int = 15):
    """Emit the per-core SPMD program. Returns compiled nc."""
    nc = bacc.Bacc("TRN2", target_bir_lowering=False, debug=False)

    # ---- DRAM I/O -------------------------------------------------------
    h16 = nc.dram_tensor("h16", [d.HID, d.BS], BF16, kind="ExternalInput")
    # ktile-major packed bf16 weights
    wq16 = nc.dram_tensor("wq16", [128, d.KT * 256], BF16,
                          kind="ExternalInput")
    wkv16 = nc.dram_tensor("wkv16", [128, d.KT * 128], BF16,
                           kind="ExternalInput")
    wo = nc.dram_tensor("wo", [2, 128, d.OUT], BF16, kind="ExternalInput")
    emat = nc.dram_tensor("emat", [128, 256], F32R, kind="ExternalInput")
    id66 = nc.dram_tensor("id66", [66, 66], BF16, kind="ExternalInput")
    vones = nc.dram_tensor("vones", [2, d.BS], BF16, kind="ExternalInput")
    rzero = nc.dram_tensor("rzero", [128, 512], F32R, kind="ExternalInput")
    out = nc.dram_tensor("out", [d.BS, d.OUT], BF16, kind="ExternalOutput")

    NQS = d.TOK_CHUNK // 128            # 4 o_proj token groups per unit
    NNH = d.OUT // 1024                 # o_proj 1024-wide col blocks

    with tile.TileContext(nc) as tc:
        with (
            tc.tile_pool(name="consts", bufs=1) as consts,
            tc.tile_pool(name="persist", bufs=1) as persist,
            tc.tile_pool(name="ht", bufs=8) as ht_pool,
            tc.tile_pool(name="expp", bufs=exp_bufs) as exp_pool,
            tc.tile_pool(name="norm", bufs=2) as norm_pool,
            tc.tile_pool(name="ostage", bufs=6) as ostage_pool,
            tc.tile_pool(name="vt", bufs=2) as vt_pool,
        ):
            # ---- constants/weights in SBUF ------------------------------
            wq_sb = consts.tile([128, d.KT * 256], BF16, tag="wq")
            wkv_sb = consts.tile([128, d.KT * 128], BF16, tag="wkv")
            wo_sb = consts.tile([128, 2 * d.OUT], BF16, tag="wo")
            emat_sb = consts.tile([128, 256], F32R, tag="emat")
            id66_sb = consts.tile([66, 66], BF16, tag="id66")
            nc.sync.dma_start(wq_sb[:], wq16[:])
            nc.sync.dma_start(wkv_sb[:], wkv16[:])
            nc.sync.dma_start(emat_sb[:], emat[:])
            nc.sync.dma_start(id66_sb[:], id66[:])

            # ---- persistent activations --------------------------------
            # q pair-packed: [64, chunk*1024]; chunk c cols =
            # [head-even 512 | head-odd 512]
            qT_sb = [persist.tile([64, d.N_TC * 1024], BF16, tag=f"qT{p}",
                                  name=f"qT{p}") for p in range(2)]
            kT_sb = persist.tile([64, d.BS], BF16, tag="kT")
            v_sb = persist.tile([128, (d.BS // 128) * 66], BF16, tag="v")
            attnT_sb = persist.tile([128, 2 * d.TOK_CHUNK], BF16,
                                    tag="attnT")
            dn_sb = persist.tile([128, 512], F32R, tag="dn")
            nc.sync.dma_start(dn_sb[:], rzero[:])

            def emit_proj(tcx, pq_pool, kv_pool, tp_psum):
                cols = slice(tcx * d.TOK_CHUNK, (tcx + 1) * d.TOK_CHUNK)
                pq = pq_pool.tile([128, 1024], F32, tag="pq", name="pq")
                pkv = kv_pool.tile([128, 512], F32, tag="pkv", name="pkv")
                for kt in range(d.KT):
                    htt = ht_pool.tile([128, 512], BF16, name="htt")
                    nc.sync.dma_start(
                        htt[:], h16[kt * 128:(kt + 1) * 128, cols])
                    fl = dict(start=(kt == 0), stop=(kt == d.KT - 1))
                    for m in range(2):
                        nc.tensor.matmul(
                            pq[:, m * 512:(m + 1) * 512],
                            wq_sb[:, kt * 256 + m * 128:
                                  kt * 256 + (m + 1) * 128],
                            htt[:], **fl)
                    nc.tensor.matmul(
                        pkv[:], wkv_sb[:, kt * 128:(kt + 1) * 128], htt[:],
                        **fl)
                # copies out (bf16)
                for p in range(2):
                    for hh in range(2):
                        nc.vector.tensor_copy(
                            qT_sb[p][:, tcx * 1024 + hh * 512:
                                     tcx * 1024 + (hh + 1) * 512],
                            pq[64 * hh:64 * hh + 64, p * 512:(p + 1) * 512])
                nc.vector.tensor_copy(kT_sb[:, cols], pkv[0:64, :])
                vt = vt_pool.tile([66, 512], BF16, name="vt")
                nc.vector.tensor_copy(vt[0:64, :], pkv[64:128, :])
                nc.sync.dma_start(vt[64:66, :], vones[:, cols])
                for j in range(d.TOK_CHUNK // 128):
                    blk = tcx * (d.TOK_CHUNK // 128) + j
                    ptv = tp_psum.tile([128, 128], BF16, name="ptv")
                    nc.tensor.transpose(
                        ptv[0:128, 0:66],
                        vt[0:66, j * 128:(j + 1) * 128], id66_sb[:])
                    nc.vector.tensor_copy(
                        v_sb[:, blk * 66:(blk + 1) * 66], ptv[0:128, 0:66])

            def emit_scores_kb(u, kb, expT, big_psum):
                b, tcx = u["b"], u["tcx"]
                kcols = slice(b * d.S + kb * 128, b * d.S + (kb + 1) * 128)
                for p in range(2):
                    st = big_psum.tile([128, 1024], F32, tag="big", name="st")
                    for hh in range(2):
                        qcols = slice(tcx * 1024 + hh * 512,
                                      tcx * 1024 + (hh + 1) * 512)
                        nc.tensor.matmul(
                            st[:, 512 * hh:512 * hh + 512],
                            kT_sb[:, kcols], qT_sb[p][:, qcols],
                            start=True, stop=True)
                    et = exp_pool.tile([128, 1024], BF16, name="et")
                    nc.scalar.activation(
                        et[:], st[:], mybir.ActivationFunctionType.Exp)
                    expT[kb, p] = et

            def emit_pv_kb(u, kb, expT):
                b = u["b"]
                vblk = (b * d.S) // 128 + kb
                for p in range(2):
                    for hh in range(2):
                        nc.tensor.matmul(
                            u["attnT_ps"][p][:, 512 * hh:512 * hh + 512],
                            v_sb[:, vblk * 66:vblk * 66 + 65],
                            expT[kb, p][:, 512 * hh:512 * hh + 512],
                            start=(kb == 0), stop=(kb == d.N_KB - 1))

            def emit_dn(u):
                # denominators -> dn rows 0/32/64/96 (head 2p+hh -> 32*(2p+hh))
                attnT_ps = u["attnT_ps"]
                for p in range(2):
                    for hh in range(2):
                        nc.vector.tensor_copy(
                            dn_sb[32 * (2 * p + hh):32 * (2 * p + hh) + 1, :],
                            attnT_ps[p][64:65, 512 * hh:512 * hh + 512])

            def emit_norm(u, big_psum):
                attnT_ps = u["attnT_ps"]
                bc_ps = big_psum.tile([128, 1024], F32, tag="big",
                                      name="bc_ps")
                bc_sb = norm_pool.tile([128, 1024], F32, name="bc_sb")
                for p in range(2):
                    nc.tensor.matmul(
                        bc_ps[:, 512 * p:512 * p + 512],
                        emat_sb[:, 128 * p:128 * (p + 1)],
                        dn_sb[:], start=True, stop=True)
                nc.vector.reciprocal_approx_fast(out=bc_sb[:], in_=bc_ps[:])
                for p in range(2):
                    for hh in range(2):
                        nc.vector.tensor_mul(
                            attnT_sb[64 * hh:64 * hh + 64,
                                     p * d.TOK_CHUNK:(p + 1) * d.TOK_CHUNK],
                            attnT_ps[p][0:64, 512 * hh:512 * hh + 512],
                            bc_sb[64 * hh:64 * hh + 64,
                                  512 * p:512 * p + 512])

            def emit_o(u, big_psum, qs_list):
                b, qc = u["b"], u["qc"]
                for qs in qs_list:
                    rows = slice(b * d.S + qc * d.TOK_CHUNK + qs * 128,
                                 b * d.S + qc * d.TOK_CHUNK + (qs + 1) * 128)
                    for nh in range(NNH):
                        po = big_psum.tile([128, 1024], F32, tag="big",
                                           name="po")
                        for ct in range(2):
                            for nn in range(2):
                                nc.tensor.matmul(
                                    po[:, nn * 512:(nn + 1) * 512],
                                    attnT_sb[:, ct * d.TOK_CHUNK + qs * 128:
                                             ct * d.TOK_CHUNK
                                             + (qs + 1) * 128],
                                    wo_sb[:, ct * d.OUT + nh * 1024
                                          + nn * 512:
                                          ct * d.OUT + nh * 1024
                                          + (nn + 1) * 512],
                                    start=(ct == 0), stop=(ct == 1))
                        ot = ostage_pool.tile([128, 1024], BF16, name="ot")
                        nc.vector.tensor_copy(ot[:], po[:])
                        nc.sync.dma_start(
                            out[rows, nh * 1024:(nh + 1) * 1024], ot[:])

            for _rep in range(repeat):
                # ---- projection phase (all chunks up front) -------------
                with (
                    tc.tile_pool(name="pqp", bufs=2, space="PSUM") as pq_pool,
                    tc.tile_pool(name="kvp", bufs=2, space="PSUM") as kv_pool,
                    tc.tile_pool(name="tpp", bufs=2, space="PSUM") as tp_psum,
                ):
                    for tcx in range(d.N_TC):
                        emit_proj(tcx, pq_pool, kv_pool, tp_psum)
                        if tcx == 0 and _rep == 0:
                            nc.sync.dma_start(
                                wo_sb[:, 0:d.OUT], wo[0, :, :])
                            nc.sync.dma_start(
                                wo_sb[:, d.OUT:2 * d.OUT], wo[1, :, :])

                # ---- attention phase ------------------------------------
                with (
                    tc.tile_pool(name="big", bufs=2, space="PSUM") as big_psum,
                    tc.tile_pool(name="attn", bufs=2, space="PSUM") as attn_ps,
                ):
                    units = []
                    for b_ in range(d.B):
                        for qc in range(d.N_QC):
                            units.append({
                                "b": b_, "qc": qc,
                                "tcx": b_ * d.N_QC + qc,
                            })
                    # o_proj(prev) token-groups spread over sbs
                    if d.NSB >= 6:
                        o_sbs = {2: [0], 3: [1], 4: [2], 5: [3]}
                    else:
                        o_sbs = {1: list(range(NQS))}
                    prev = None
                    for u in units:
                        expT = {}
                        LAG = 4 if d.NSB >= 4 else 2
                        for sb in range(d.NSB):
                            for kb in range(2 * sb, 2 * sb + 2):
                                emit_scores_kb(u, kb, expT, big_psum)
                                if kb >= LAG:
                                    if u.get("attnT_ps") is None:
                                        u["attnT_ps"] = [
                                            attn_ps.tile(
                                                [65, 1024], F32, tag="at",
                                                name=f"attnT_ps{p}")
                                            for p in range(2)]
                                    emit_pv_kb(u, kb - LAG, expT)
                            if sb == 0 and prev is not None:
                                emit_norm(prev, big_psum)
                            if prev is not None and sb in o_sbs:
                                emit_o(prev, big_psum, o_sbs[sb])
                        for kb in range(d.N_KB - LAG, d.N_KB):
                            emit_pv_kb(u, kb, expT)
                        emit_dn(u)
                        prev = u
                    emit_norm(prev, big_psum)
                    emit_o(prev, big_psum, list(range(NQS)))

    nc.compile()
    return nc


def _rope_fold(W, cos, sin, nheads, scale):
    """Fold head-constant RoPE (and scale) into a projection weight.
    W: [HID, nheads*64] fp32; cos/sin: [nheads, 64]."""
    W4 = W.reshape(W.shape[0], nheads, 64)
    out = np.empty_like(W4)
    out[:, :, :32] = W4[:, :, :32] * cos[None, :, :32] \
        - W4[:, :, 32:] * sin[None, :, :32]
    out[:, :, 32:] = W4[:, :, 32:] * cos[None, :, 32:] \
        + W4[:, :, :32] * sin[None, :, 32:]
    return (out * scale).reshape(W.shape)


def _pack_w16(W):
    """W [HID, M] fp32 -> ktile-major [128, KT*M] bf16."""
    HIDd, M = W.shape
    return np.ascontiguousarray(
        W.reshape(HIDd // 128, 128, M).transpose(1, 0, 2)
        .reshape(128, -1).astype(bf16))


_PROGRAM_CACHE = {}


def _get_program():
    if "nc" not in _PROGRAM_CACHE:
        _PROGRAM_CACHE["nc"] = build_program(Dims())
    return _PROGRAM_CACHE["nc"]


def make_in_maps(hidden_states, Wq, Wk, Wv, Wo, cos, sin, d: Dims = None):
    """Host-side sharding/prep. Returns per-core input dicts."""
    d = d or Dims()
    hs = np.asarray(hidden_states, np.float32).reshape(d.BS, d.HID)
    hT = np.ascontiguousarray(hs.T.astype(bf16))
    cos = np.asarray(cos, np.float32)
    sin = np.asarray(sin, np.float32)
    nq_total = N_CORES * d.NQ
    Wq_f = _rope_fold(np.asarray(Wq, np.float32), cos[:nq_total],
                      sin[:nq_total], nq_total, 1.0 / np.sqrt(d.HD))
    Wk_f = _rope_fold(np.asarray(Wk, np.float32), cos[:KVH], sin[:KVH],
                      KVH, 1.0)
    Wv_f = np.asarray(Wv, np.float32)
    Wo_f = np.asarray(Wo, np.float32)
    emat = np.zeros([128, 256], np.float32)
    for h in range(4):
        p, hh = h // 2, h % 2
        emat[32 * h, 128 * p + 64 * hh:128 * p + 64 * hh + 64] = 1.0
    id66 = np.eye(66, dtype=bf16)
    vones = np.concatenate([np.ones([1, d.BS], bf16),
                            np.zeros([1, d.BS], bf16)])
    rzero = np.zeros([128, 512], np.float32)
    in_maps = []
    for c in range(N_CORES):
        wq_c = _pack_w16(Wq_f[:, c * d.QCH:(c + 1) * d.QCH])
        wkv_c = _pack_w16(np.concatenate(
            [Wk_f[:, c * d.HD:(c + 1) * d.HD],
             Wv_f[:, c * d.HD:(c + 1) * d.HD]], axis=1))
        wo_c = np.ascontiguousarray(
            Wo_f[c * d.QCH:(c + 1) * d.QCH, :].reshape(2, 128, d.OUT)
            .astype(bf16))
        in_maps.append({
            "h16": hT, "wq16": wq_c, "wkv16": wkv_c,
            "wo": wo_c, "emat": emat, "id66": id66, "vones": vones,
            "rzero": rzero,
        })
    return in_maps


def kernel(hidden_states, Wq, Wk, Wv, Wo, cos, sin):
    d = Dims()
    nc = _get_program()
    in_maps = make_in_maps(hidden_states, Wq, Wk, Wv, Wo, cos, sin, d)
    res = bass_utils.run_bass_kernel_spmd(
        nc, in_maps, core_ids=list(range(N_CORES)))
    acc = res.results[0]["out"].astype(np.float32)
    for c in range(1, N_CORES):
        acc += res.results[c]["out"].astype(np.float32)
    return acc.reshape(B, S, HID)


if __name__ == "__main__":
    rng = np.random.default_rng(0)
    h = rng.standard_normal((B, S, HID), dtype=np.float32)
    sc = 1.0 / np.sqrt(HID)
    Wq_ = rng.standard_normal((HID, NH * HD), dtype=np.float32) * sc
    Wk_ = rng.standard_normal((HID, KVH * HD), dtype=np.float32) * sc
    Wv_ = rng.standard_normal((HID, KVH * HD), dtype=np.float32) * sc
    Wo_ = rng.standard_normal((NH * HD, HID), dtype=np.float32) * sc
    inv = 1.0 / (10000.0 ** (np.arange(0, HD, 2, dtype=np.float32) / HD))
    t = np.arange(S, dtype=np.float32)
    fr = np.outer(t, inv)
    emb = np.concatenate([fr, fr], axis=-1)
    o = kernel(h, Wq_, Wk_, Wv_, Wo_, np.cos(emb), np.sin(emb))
    print("out", o.shape, o.dtype, float(np.abs(o).max()))


# revision 25
# speedup vs baseline: 1.9382x; 1.9382x over previous
"""Trainium2 Bass kernel for nn_AvaAttention (GQA attention, head-constant RoPE).

Sharding: tensor-parallel over the 8 kv heads -> core c owns kv head c and
q heads 4c..4c+3. Each core computes its 4 heads' attention and a partial
o_proj (row-split Wo); the host sums the 8 partials.

v2 design (per-core):
- RoPE is head-constant (the module indexes cos/sin by head), so it folds
  into Wq/Wk on the host along with the 1/sqrt(D) scale.
- The whole pipeline runs in bf16 (hidden states DMA'd as bf16 -> half
  the input traffic of the fp32 baseline; q/k/v, exp probs, attn and Wo
  all bf16). Measured end-to-end error ~7e-3, well under the 2e-2 gate;
  PE cost for bf16 is the same 1 col/cycle as fp32r.
  (fp8 DoubleRow + hi/lo residual projections were tried and work
  numerically, but each non-fp32 matmul is split into Ldweights+Matmult,
  and at 107ns/MM the extra ~71ns sequencer dispatch makes the PE
  dispatch-bound - no wall-clock win, so bf16 was kept.)
- Scores are computed transposed ([ktok, qtok]) so exp feeds the PV matmul
  directly; a 65th all-ones column of v makes the PV matmul emit the
  softmax denominator (row 64). exp runs on ScalarE ([128,1024] tiles,
  one per (kb, head-pair)).
- q is stored pair-packed ([64, chunk*2*512]: head-even cols then head-odd
  cols per chunk) so scores need no partition-offset staging.
- Softmax normalization is deferred: denominators are broadcast via a
  tiny PE matmul (emat), reciprocaled on DVE, and multiplied into
  attnT_sb (bf16) which feeds o_proj.
- Projections run up front (8 chunks, PE-bound with fp8 DMA); the 8
  attention units are software-pipelined: unit u's scores/exp overlap
  unit u-1's PV normalize + o_proj.
- PSUM: scores/bcast/po share a 2x[128,1024] ring (4 banks); the two
  per-unit attnT accumulators [65,1024] live in a bufs=2 pool (4 banks);
  projections use their own pools inside a context that exits before the
  attention pools open.
"""

import numpy as np
import ml_dtypes

import concourse.bass as bass
import concourse.bacc as bacc
import concourse.tile as tile
import concourse.mybir as mybir
from concourse import bass_utils

BF16 = mybir.dt.bfloat16
F32 = mybir.dt.float32
F32R = mybir.dt.float32r
FP8 = mybir.dt.float8e4
DR = mybir.MatmulPerfMode.DoubleRow
bf16 = ml_dtypes.bfloat16
e4m3 = ml_dtypes.float8_e4m3

# Problem dims (hardcoded per contract)
B, S, HID = 2, 2048, 2048
NH, KVH, HD = 32, 8, 64
N_CORES = 8


class Dims:
    """All derived tile counts; parameterized so small variants can be
    simulated in CoreSim."""

    def __init__(self, B=B, S=S, HID=HID, n_qheads=4, HD=HD, out_ch=HID):
        self.B, self.S, self.HID, self.HD = B, S, HID, HD
        self.NQ = n_qheads              # q heads per core (must be 4)
        self.BS = B * S                 # total tokens
        self.QCH = n_qheads * HD        # q channels per core (256)
        self.OUT = out_ch               # o_proj output channels
        self.TOK_CHUNK = 512            # projection/attention token chunk
        self.KT = HID // 128            # contraction tiles for projections
        self.NPAIR = self.KT // 2       # DoubleRow k-tile pairs
        self.N_TC = self.BS // self.TOK_CHUNK
        self.N_QC = S // self.TOK_CHUNK  # q chunks per batch
        self.N_KB = S // 128            # ktok blocks per batch
        self.NSB = self.N_KB // 2
        assert n_qheads == 4 and HD == 64
        assert self.KT % 2 == 0 and self.OUT % 1024 == 0 and S % 512 == 0


def build_program(d: Dims, repeat: int = 1, exp_bufs: int = 15):
    """Emit the per-core SPMD program. Returns compiled nc."""
    nc = bacc.Bacc("TRN2", target_bir_lowering=False, debug=False)

    # ---- DRAM I/O -------------------------------------------------------
    h16 = nc.dram_tensor("h16", [d.HID, d.BS], BF16, kind="ExternalInput")
    # ktile-major packed bf16 weights
    wq16 = nc.dram_tensor("wq16", [128, d.KT * 256], BF16,
                          kind="ExternalInput")
    wkv16 = nc.dram_tensor("wkv16", [128, d.KT * 128], BF16,
                           kind="ExternalInput")
    wo = nc.dram_tensor("wo", [2, 128, d.OUT], BF16, kind="ExternalInput")
    emat = nc.dram_tensor("emat", [128, 256], F32R, kind="ExternalInput")
    id66 = nc.dram_tensor("id66", [66, 66], BF16, kind="ExternalInput")
    vones = nc.dram_tensor("vones", [2, d.BS], BF16, kind="ExternalInput")
    rzero = nc.dram_tensor("rzero", [128, 512], F32R, kind="ExternalInput")
    out = nc.dram_tensor("out", [d.BS, d.OUT], BF16, kind="ExternalOutput")

    NQS = d.TOK_CHUNK // 128            # 4 o_proj token groups per unit
    NNH = d.OUT // 1024                 # o_proj 1024-wide col blocks

    with tile.TileContext(nc) as tc:
        with (
            tc.tile_pool(name="consts", bufs=1) as consts,
            tc.tile_pool(name="persist", bufs=1) as persist,
            tc.tile_pool(name="ht", bufs=8) as ht_pool,
            tc.tile_pool(name="expp", bufs=exp_bufs) as exp_pool,
            tc.tile_pool(name="norm", bufs=2) as norm_pool,
            tc.tile_pool(name="ostage", bufs=6) as ostage_pool,
            tc.tile_pool(name="vt", bufs=2) as vt_pool,
        ):
            # ---- constants/weights in SBUF ------------------------------
            wq_sb = consts.tile([128, d.KT * 256], BF16, tag="wq")
            wkv_sb = consts.tile([128, d.KT * 128], BF16, tag="wkv")
            wo_sb = consts.tile([128, 2 * d.OUT], BF16, tag="wo")
            emat_sb = consts.tile([128, 256], F32R, tag="emat")
            id66_sb = consts.tile([66, 66], BF16, tag="id66")
            # stage weight DMAs so the first proj matmul waits only the
            # first quarter of wq; the rest land between P-iterations of
            # chunk 0 (emission-ordered ahead of their first reader)
            if d.KT == 16:
                wq_q = [(wq_sb[:, i * 1024:(i + 1) * 1024],
                         wq16[:, i * 1024:(i + 1) * 1024]) for i in range(4)]
                wkv_h = [(wkv_sb[:, i * 1024:(i + 1) * 1024],
                          wkv16[:, i * 1024:(i + 1) * 1024])
                         for i in range(2)]
                nc.sync.dma_start(*wq_q[0])
                nc.sync.dma_start(*wkv_h[0])
                wpieces = [wq_q[1], wkv_h[1], wq_q[2], wq_q[3]]
            else:
                nc.sync.dma_start(wq_sb[:], wq16[:])
                nc.sync.dma_start(wkv_sb[:], wkv16[:])
                wpieces = []
            nc.sync.dma_start(emat_sb[:], emat[:])
            nc.sync.dma_start(id66_sb[:], id66[:])

            # ---- persistent activations --------------------------------
            # q pair-packed: [64, chunk*1024]; chunk c cols =
            # [head-even 512 | head-odd 512]
            qT_sb = [persist.tile([64, d.N_TC * 1024], BF16, tag=f"qT{p}",
                                  name=f"qT{p}") for p in range(2)]
            kT_sb = persist.tile([64, d.BS], BF16, tag="kT")
            v_sb = persist.tile([128, (d.BS // 128) * 66], BF16, tag="v")
            attnT_sb = persist.tile([128, 2 * d.TOK_CHUNK], BF16,
                                    tag="attnT")
            dn_sb = persist.tile([128, 512], F32R, tag="dn")
            nc.sync.dma_start(dn_sb[:], rzero[:])

            def emit_proj(tcx, pq_pool, kv_pool, tp_psum):
                cols = slice(tcx * d.TOK_CHUNK, (tcx + 1) * d.TOK_CHUNK)
                pq = pq_pool.tile([128, 1024], F32, tag="pq", name="pq")
                pkv = kv_pool.tile([128, 512], F32, tag="pkv", name="pkv")
                # one [128,1024] piece per 256-row block: col-half j holds
                # the block's rows {2p+j} (row-major DMA flatten), matching
                # the host's even/odd-interleaved W packing
                for P in range(d.KT // 2):
                    if tcx == 0 and 1 <= P <= len(wpieces):
                        nc.sync.dma_start(*wpieces[P - 1])
                    htt = ht_pool.tile([128, 1024], BF16, name="htt")
                    nc.sync.dma_start(
                        htt[:], h16[P * 256:(P + 1) * 256, cols])
                    for j in range(2):
                        kt = 2 * P + j
                        fl = dict(start=(kt == 0), stop=(kt == d.KT - 1))
                        for m in range(2):
                            nc.tensor.matmul(
                                pq[:, m * 512:(m + 1) * 512],
                                wq_sb[:, kt * 256 + m * 128:
                                      kt * 256 + (m + 1) * 128],
                                htt[:, j * 512:(j + 1) * 512], **fl)
                        nc.tensor.matmul(
                            pkv[:], wkv_sb[:, kt * 128:(kt + 1) * 128],
                            htt[:, j * 512:(j + 1) * 512], **fl)
                # copies out (bf16)
                for p in range(2):
                    for hh in range(2):
                        nc.vector.tensor_copy(
                            qT_sb[p][:, tcx * 1024 + hh * 512:
                                     tcx * 1024 + (hh + 1) * 512],
                            pq[64 * hh:64 * hh + 64, p * 512:(p + 1) * 512])
                nc.vector.tensor_copy(kT_sb[:, cols], pkv[0:64, :])
                vt = vt_pool.tile([66, 512], BF16, name="vt")
                nc.vector.tensor_copy(vt[0:64, :], pkv[64:128, :])
                nc.sync.dma_start(vt[64:66, :], vones[:, cols])
                for j in range(d.TOK_CHUNK // 128):
                    blk = tcx * (d.TOK_CHUNK // 128) + j
                    ptv = tp_psum.tile([128, 128], BF16, name="ptv")
                    nc.tensor.transpose(
                        ptv[0:128, 0:66],
                        vt[0:66, j * 128:(j + 1) * 128], id66_sb[:])
                    nc.vector.tensor_copy(
                        v_sb[:, blk * 66:(blk + 1) * 66], ptv[0:128, 0:66])

            def emit_scores_kb(u, kb, expT, big_psum):
                b, tcx = u["b"], u["tcx"]
                kcols = slice(b * d.S + kb * 128, b * d.S + (kb + 1) * 128)
                for p in range(2):
                    st = big_psum.tile([128, 1024], F32, tag="big", name="st")
                    for hh in range(2):
                        qcols = slice(tcx * 1024 + hh * 512,
                                      tcx * 1024 + (hh + 1) * 512)
                        nc.tensor.matmul(
                            st[:, 512 * hh:512 * hh + 512],
                            kT_sb[:, kcols], qT_sb[p][:, qcols],
                            start=True, stop=True)
                    et = exp_pool.tile([128, 1024], BF16, name="et")
                    nc.scalar.activation(
                        et[:], st[:], mybir.ActivationFunctionType.Exp)
                    expT[kb, p] = et

            def emit_pv_kb(u, kb, expT):
                b = u["b"]
                vblk = (b * d.S) // 128 + kb
                for p in range(2):
                    for hh in range(2):
                        nc.tensor.matmul(
                            u["attnT_ps"][p][:, 512 * hh:512 * hh + 512],
                            v_sb[:, vblk * 66:vblk * 66 + 65],
                            expT[kb, p][:, 512 * hh:512 * hh + 512],
                            start=(kb == 0), stop=(kb == d.N_KB - 1))

            def emit_dn(u):
                # denominators -> dn rows 0/32/64/96 (head 2p+hh -> 32*(2p+hh))
                attnT_ps = u["attnT_ps"]
                for p in range(2):
                    for hh in range(2):
                        nc.vector.tensor_copy(
                            dn_sb[32 * (2 * p + hh):32 * (2 * p + hh) + 1, :],
                            attnT_ps[p][64:65, 512 * hh:512 * hh + 512])

            def emit_norm(u, big_psum):
                attnT_ps = u["attnT_ps"]
                bc_ps = big_psum.tile([128, 1024], F32, tag="big",
                                      name="bc_ps")
                bc_sb = norm_pool.tile([128, 1024], F32, name="bc_sb")
                for p in range(2):
                    nc.tensor.matmul(
                        bc_ps[:, 512 * p:512 * p + 512],
                        emat_sb[:, 128 * p:128 * (p + 1)],
                        dn_sb[:], start=True, stop=True)
                nc.vector.reciprocal_approx_fast(out=bc_sb[:], in_=bc_ps[:])
                for p in range(2):
                    for hh in range(2):
                        nc.vector.tensor_mul(
                            attnT_sb[64 * hh:64 * hh + 64,
                                     p * d.TOK_CHUNK:(p + 1) * d.TOK_CHUNK],
                            attnT_ps[p][0:64, 512 * hh:512 * hh + 512],
                            bc_sb[64 * hh:64 * hh + 64,
                                  512 * p:512 * p + 512])

            def emit_o(u, big_psum, qs_list):
                b, qc = u["b"], u["qc"]
                for qs in qs_list:
                    rows = slice(b * d.S + qc * d.TOK_CHUNK + qs * 128,
                                 b * d.S + qc * d.TOK_CHUNK + (qs + 1) * 128)
                    for nh in range(NNH):
                        po = big_psum.tile([128, 1024], F32, tag="big",
                                           name="po")
                        for ct in range(2):
                            for nn in range(2):
                                nc.tensor.matmul(
                                    po[:, nn * 512:(nn + 1) * 512],
                                    attnT_sb[:, ct * d.TOK_CHUNK + qs * 128:
                                             ct * d.TOK_CHUNK
                                             + (qs + 1) * 128],
                                    wo_sb[:, ct * d.OUT + nh * 1024
                                          + nn * 512:
                                          ct * d.OUT + nh * 1024
                                          + (nn + 1) * 512],
                                    start=(ct == 0), stop=(ct == 1))
                        ot = ostage_pool.tile([128, 1024], BF16, name="ot")
                        nc.vector.tensor_copy(ot[:], po[:])
                        nc.sync.dma_start(
                            out[rows, nh * 1024:(nh + 1) * 1024], ot[:])

            for _rep in range(repeat):
                # ---- projection phase (all chunks up front) -------------
                with (
                    tc.tile_pool(name="pqp", bufs=2, space="PSUM") as pq_pool,
                    tc.tile_pool(name="kvp", bufs=2, space="PSUM") as kv_pool,
                    tc.tile_pool(name="tpp", bufs=2, space="PSUM") as tp_psum,
                ):
                    for tcx in range(d.N_TC):
                        emit_proj(tcx, pq_pool, kv_pool, tp_psum)
                        if tcx == 0 and _rep == 0:
                            nc.sync.dma_start(
                                wo_sb[:, 0:d.OUT], wo[0, :, :])
                            nc.sync.dma_start(
                                wo_sb[:, d.OUT:2 * d.OUT], wo[1, :, :])

                # ---- attention phase ------------------------------------
                with (
                    tc.tile_pool(name="big", bufs=2, space="PSUM") as big_psum,
                    tc.tile_pool(name="attn", bufs=2, space="PSUM") as attn_ps,
                ):
                    units = []
                    for b_ in range(d.B):
                        for qc in range(d.N_QC):
                            units.append({
                                "b": b_, "qc": qc,
                                "tcx": b_ * d.N_QC + qc,
                            })
                    # o_proj(prev) token-groups spread over sbs
                    if d.NSB >= 6:
                        o_sbs = {3: [0], 4: [1], 5: [2], 6: [3]}
                    else:
                        o_sbs = {1: list(range(NQS))}
                    prev = None
                    for u in units:
                        expT = {}
                        LAG = 4 if d.NSB >= 4 else 2
                        for sb in range(d.NSB):
                            for kb in range(2 * sb, 2 * sb + 2):
                                emit_scores_kb(u, kb, expT, big_psum)
                                if kb >= LAG:
                                    if u.get("attnT_ps") is None:
                                        u["attnT_ps"] = [
                                            attn_ps.tile(
                                                [65, 1024], F32, tag="at",
                                                name=f"attnT_ps{p}")
                                            for p in range(2)]
                                    emit_pv_kb(u, kb - LAG, expT)
                            if sb == 0 and prev is not None:
                                emit_norm(prev, big_psum)
                            if prev is not None and sb in o_sbs:
                                emit_o(prev, big_psum, o_sbs[sb])
                        for kb in range(d.N_KB - LAG, d.N_KB):
                            emit_pv_kb(u, kb, expT)
                        emit_dn(u)
                        prev = u
                    emit_norm(prev, big_psum)
                    emit_o(prev, big_psum, list(range(NQS)))

    nc.compile()
    return nc


def _rope_fold(W, cos, sin, nheads, scale):
    """Fold head-constant RoPE (and scale) into a projection weight.
    W: [HID, nheads*64] fp32; cos/sin: [nheads, 64]."""
    W4 = W.reshape(W.shape[0], nheads, 64)
    out = np.empty_like(W4)
    out[:, :, :32] = W4[:, :, :32] * cos[None, :, :32] \
        - W4[:, :, 32:] * sin[None, :, :32]
    out[:, :, 32:] = W4[:, :, 32:] * cos[None, :, 32:] \
        + W4[:, :, :32] * sin[None, :, 32:]
    return (out * scale).reshape(W.shape)


def _pack_w16(W):
    """W [HID, M] fp32 -> ktile-major [128, KT*M] bf16, where virtual
    ktile 2P+j holds rows {P*256 + 2p + j} (the row order produced by
    DMA-flattening a [256, tok] h slice into a [128, 1024] tile)."""
    HIDd, M = W.shape
    Wp = W.reshape(HIDd // 256, 128, 2, M).transpose(2, 0, 1, 3) \
          .reshape(2, HIDd // 256, 128, M)
    # virtual tile (P, j) = Wp[j, P]; order tiles as 2P+j
    tiles = [Wp[j, P] for P in range(HIDd // 256) for j in range(2)]
    return np.ascontiguousarray(
        np.stack(tiles, 0).transpose(1, 0, 2).reshape(128, -1).astype(bf16))


_PROGRAM_CACHE = {}


def _get_program():
    if "nc" not in _PROGRAM_CACHE:
        _PROGRAM_CACHE["nc"] = build_program(Dims())
    return _PROGRAM_CACHE["nc"]


def make_in_maps(hidden_states, Wq, Wk, Wv, Wo, cos, sin, d: Dims = None):
    """Host-side sharding/prep. Returns per-core input dicts."""
    d = d or Dims()
    hs = np.asarray(hidden_states, np.float32).reshape(d.BS, d.HID)
    hT = np.ascontiguousarray(hs.T.astype(bf16))
    cos = np.asarray(cos, np.float32)
    sin = np.asarray(sin, np.float32)
    nq_total = N_CORES * d.NQ
    Wq_f = _rope_fold(np.asarray(Wq, np.float32), cos[:nq_total],
                      sin[:nq_total], nq_total, 1.0 / np.sqrt(d.HD))
    Wk_f = _rope_fold(np.asarray(Wk, np.float32), cos[:KVH], sin[:KVH],
                      KVH, 1.0)
    Wv_f = np.asarray(Wv, np.float32)
    Wo_f = np.asarray(Wo, np.float32)
    emat = np.zeros([128, 256], np.float32)
    for h in range(4):
        p, hh = h // 2, h % 2
        emat[32 * h, 128 * p + 64 * hh:128 * p + 64 * hh + 64] = 1.0
    id66 = np.eye(66, dtype=bf16)
    vones = np.concatenate([np.ones([1, d.BS], bf16),
                            np.zeros([1, d.BS], bf16)])
    rzero = np.zeros([128, 512], np.float32)
    in_maps = []
    for c in range(N_CORES):
        wq_c = _pack_w16(Wq_f[:, c * d.QCH:(c + 1) * d.QCH])
        wkv_c = _pack_w16(np.concatenate(
            [Wk_f[:, c * d.HD:(c + 1) * d.HD],
             Wv_f[:, c * d.HD:(c + 1) * d.HD]], axis=1))
        wo_c = np.ascontiguousarray(
            Wo_f[c * d.QCH:(c + 1) * d.QCH, :].reshape(2, 128, d.OUT)
            .astype(bf16))
        in_maps.append({
            "h16": hT, "wq16": wq_c, "wkv16": wkv_c,
            "wo": wo_c, "emat": emat, "id66": id66, "vones": vones,
            "rzero": rzero,
        })
    return in_maps


def kernel(hidden_states, Wq, Wk, Wv, Wo, cos, sin):
    d = Dims()
    nc = _get_program()
    in_maps = make_in_maps(hidden_states, Wq, Wk, Wv, Wo, cos, sin, d)
    res = bass_utils.run_bass_kernel_spmd(
        nc, in_maps, core_ids=list(range(N_CORES)))
    acc = res.results[0]["out"].astype(np.float32)
    for c in range(1, N_CORES):
        acc += res.results[c]["out"].astype(np.float32)
    return acc.reshape(B, S, HID)


if __name__ == "__main__":
    rng = np.random.default_rng(0)
    h = rng.standard_normal((B, S, HID), dtype=np.float32)
    sc = 1.0 / np.sqrt(HID)
    Wq_ = rng.standard_normal((HID, NH * HD), dtype=np.float32) * sc
    Wk_ = rng.standard_normal((HID, KVH * HD), dtype=np.float32) * sc
    Wv_ = rng.standard_normal((HID, KVH * HD), dtype=np.float32) * sc
    Wo_ = rng.standard_normal((NH * HD, HID), dtype=np.float32) * sc
    inv = 1.0 / (10000.0 ** (np.arange(0, HD, 2, dtype=np.float32) / HD))
    t = np.arange(S, dtype=np.float32)
    fr = np.outer(t, inv)
    emb = np.concatenate([fr, fr], axis=-1)
    o = kernel(h, Wq_, Wk_, Wv_, Wo_, np.cos(emb), np.sin(emb))
    print("out", o.shape, o.dtype, float(np.abs(o).max()))


# revision 28
# speedup vs baseline: 1.9422x; 1.0020x over previous
"""Trainium2 Bass kernel for nn_AvaAttention (GQA attention, head-constant RoPE).

Sharding: tensor-parallel over the 8 kv heads -> core c owns kv head c and
q heads 4c..4c+3. Each core computes its 4 heads' attention and a partial
o_proj (row-split Wo); the host sums the 8 partials.

v2 design (per-core):
- RoPE is head-constant (the module indexes cos/sin by head), so it folds
  into Wq/Wk on the host along with the 1/sqrt(D) scale.
- The whole pipeline runs in bf16 (hidden states DMA'd as bf16 -> half
  the input traffic of the fp32 baseline; q/k/v, exp probs, attn and Wo
  all bf16). Measured end-to-end error ~7e-3, well under the 2e-2 gate;
  PE cost for bf16 is the same 1 col/cycle as fp32r.
  (fp8 DoubleRow + hi/lo residual projections were tried and work
  numerically, but each non-fp32 matmul is split into Ldweights+Matmult,
  and at 107ns/MM the extra ~71ns sequencer dispatch makes the PE
  dispatch-bound - no wall-clock win, so bf16 was kept.)
- Scores are computed transposed ([ktok, qtok]) so exp feeds the PV matmul
  directly; a 65th all-ones column of v makes the PV matmul emit the
  softmax denominator (row 64). exp runs on ScalarE ([128,1024] tiles,
  one per (kb, head-pair)).
- q is stored pair-packed ([64, chunk*2*512]: head-even cols then head-odd
  cols per chunk) so scores need no partition-offset staging.
- Softmax normalization is deferred: denominators are broadcast via a
  tiny PE matmul (emat), reciprocaled on DVE, and multiplied into
  attnT_sb (bf16) which feeds o_proj.
- Projections run up front, PE-bound (2 k-tiles per [128,1024] h DMA;
  the SP queue's ~600ns/DMA issue rate made 16-DMA chunks issue-bound).
  Batch-1 projections run with narrowed PSUM pools so unit 0's first 6
  score blocks (and their ScalarE exps) overlap them - ScalarE is
  otherwise idle for the whole projection phase.
- The 8 attention units are software-pipelined: unit u's scores/exp
  overlap unit u-1's PV normalize + o_proj (o_proj token-groups spread
  over sub-blocks 3..6, PV lagging scores by 4 k-blocks).
- PSUM: scores/bcast/po share a 2x[128,1024] ring (4 banks); the two
  per-unit attnT accumulators [65,1024] live in a bufs=2 pool (4 banks);
  projections use their own pools inside contexts that exit before the
  attention pools open.
"""

import numpy as np
import ml_dtypes

import concourse.bass as bass
import concourse.bacc as bacc
import concourse.tile as tile
import concourse.mybir as mybir
from concourse import bass_utils

BF16 = mybir.dt.bfloat16
F32 = mybir.dt.float32
F32R = mybir.dt.float32r
FP8 = mybir.dt.float8e4
DR = mybir.MatmulPerfMode.DoubleRow
bf16 = ml_dtypes.bfloat16
e4m3 = ml_dtypes.float8_e4m3

# Problem dims (hardcoded per contract)
B, S, HID = 2, 2048, 2048
NH, KVH, HD = 32, 8, 64
N_CORES = 8


class Dims:
    """All derived tile counts; parameterized so small variants can be
    simulated in CoreSim."""

    def __init__(self, B=B, S=S, HID=HID, n_qheads=4, HD=HD, out_ch=HID):
        self.B, self.S, self.HID, self.HD = B, S, HID, HD
        self.NQ = n_qheads              # q heads per core (must be 4)
        self.BS = B * S                 # total tokens
        self.QCH = n_qheads * HD        # q channels per core (256)
        self.OUT = out_ch               # o_proj output channels
        self.TOK_CHUNK = 512            # projection/attention token chunk
        self.KT = HID // 128            # contraction tiles for projections
        self.NPAIR = self.KT // 2       # DoubleRow k-tile pairs
        self.N_TC = self.BS // self.TOK_CHUNK
        self.N_QC = S // self.TOK_CHUNK  # q chunks per batch
        self.N_KB = S // 128            # ktok blocks per batch
        self.NSB = self.N_KB // 2
        assert n_qheads == 4 and HD == 64
        assert self.KT % 2 == 0 and self.OUT % 1024 == 0 and S % 512 == 0


def build_program(d: Dims, repeat: int = 1, exp_bufs: int = 15):
    """Emit the per-core SPMD program. Returns compiled nc."""
    nc = bacc.Bacc("TRN2", target_bir_lowering=False, debug=False)

    # ---- DRAM I/O -------------------------------------------------------
    h16 = nc.dram_tensor("h16", [d.HID, d.BS], BF16, kind="ExternalInput")
    # ktile-major packed bf16 weights
    wq16 = nc.dram_tensor("wq16", [128, d.KT * 256], BF16,
                          kind="ExternalInput")
    wkv16 = nc.dram_tensor("wkv16", [128, d.KT * 128], BF16,
                           kind="ExternalInput")
    wo = nc.dram_tensor("wo", [2, 128, d.OUT], BF16, kind="ExternalInput")
    emat = nc.dram_tensor("emat", [128, 256], F32R, kind="ExternalInput")
    id66 = nc.dram_tensor("id66", [66, 66], BF16, kind="ExternalInput")
    vones = nc.dram_tensor("vones", [2, d.BS], BF16, kind="ExternalInput")
    rzero = nc.dram_tensor("rzero", [128, 512], F32R, kind="ExternalInput")
    out = nc.dram_tensor("out", [d.BS, d.OUT], BF16, kind="ExternalOutput")

    NQS = d.TOK_CHUNK // 128            # 4 o_proj token groups per unit
    NNH = d.OUT // 1024                 # o_proj 1024-wide col blocks

    with tile.TileContext(nc) as tc:
        with (
            tc.tile_pool(name="consts", bufs=1) as consts,
            tc.tile_pool(name="persist", bufs=1) as persist,
            tc.tile_pool(name="ht", bufs=8) as ht_pool,
            tc.tile_pool(name="expp", bufs=exp_bufs) as exp_pool,
            tc.tile_pool(name="norm", bufs=2) as norm_pool,
            tc.tile_pool(name="ostage", bufs=6) as ostage_pool,
            tc.tile_pool(name="vt", bufs=2) as vt_pool,
        ):
            # ---- constants/weights in SBUF ------------------------------
            wq_sb = consts.tile([128, d.KT * 256], BF16, tag="wq")
            wkv_sb = consts.tile([128, d.KT * 128], BF16, tag="wkv")
            wo_sb = consts.tile([128, 2 * d.OUT], BF16, tag="wo")
            emat_sb = consts.tile([128, 256], F32R, tag="emat")
            id66_sb = consts.tile([66, 66], BF16, tag="id66")
            # stage weight DMAs so the first proj matmul waits only the
            # first quarter of wq; the rest land between P-iterations of
            # chunk 0 (emission-ordered ahead of their first reader)
            if d.KT == 16:
                wq_q = [(wq_sb[:, i * 1024:(i + 1) * 1024],
                         wq16[:, i * 1024:(i + 1) * 1024]) for i in range(4)]
                wkv_h = [(wkv_sb[:, i * 1024:(i + 1) * 1024],
                          wkv16[:, i * 1024:(i + 1) * 1024])
                         for i in range(2)]
                nc.sync.dma_start(*wq_q[0])
                nc.sync.dma_start(*wkv_h[0])
                wpieces = [wq_q[1], wkv_h[1], wq_q[2], wq_q[3]]
            else:
                nc.sync.dma_start(wq_sb[:], wq16[:])
                nc.sync.dma_start(wkv_sb[:], wkv16[:])
                wpieces = []
            nc.sync.dma_start(emat_sb[:], emat[:])
            nc.sync.dma_start(id66_sb[:], id66[:])

            # ---- persistent activations --------------------------------
            # q pair-packed: [64, chunk*1024]; chunk c cols =
            # [head-even 512 | head-odd 512]
            qT_sb = [persist.tile([64, d.N_TC * 1024], BF16, tag=f"qT{p}",
                                  name=f"qT{p}") for p in range(2)]
            kT_sb = persist.tile([64, d.BS], BF16, tag="kT")
            v_sb = persist.tile([128, (d.BS // 128) * 66], BF16, tag="v")
            attnT_sb = persist.tile([128, 2 * d.TOK_CHUNK], BF16,
                                    tag="attnT")
            dn_sb = persist.tile([128, 512], F32R, tag="dn")
            nc.sync.dma_start(dn_sb[:], rzero[:])

            def emit_proj(tcx, pq_pool, kv_pool, tp_psum):
                cols = slice(tcx * d.TOK_CHUNK, (tcx + 1) * d.TOK_CHUNK)
                pq = pq_pool.tile([128, 1024], F32, tag="pq", name="pq")
                pkv = kv_pool.tile([128, 512], F32, tag="pkv", name="pkv")
                # one [128,1024] piece per 256-row block: col-half j holds
                # the block's rows {2p+j} (row-major DMA flatten), matching
                # the host's even/odd-interleaved W packing
                for P in range(d.KT // 2):
                    if tcx == 0 and 1 <= P <= len(wpieces):
                        nc.sync.dma_start(*wpieces[P - 1])
                    htt = ht_pool.tile([128, 1024], BF16, name="htt")
                    nc.sync.dma_start(
                        htt[:], h16[P * 256:(P + 1) * 256, cols])
                    for j in range(2):
                        kt = 2 * P + j
                        fl = dict(start=(kt == 0), stop=(kt == d.KT - 1))
                        for m in range(2):
                            nc.tensor.matmul(
                                pq[:, m * 512:(m + 1) * 512],
                                wq_sb[:, kt * 256 + m * 128:
                                      kt * 256 + (m + 1) * 128],
                                htt[:, j * 512:(j + 1) * 512], **fl)
                        nc.tensor.matmul(
                            pkv[:], wkv_sb[:, kt * 128:(kt + 1) * 128],
                            htt[:, j * 512:(j + 1) * 512], **fl)
                # copies out (bf16)
                for p in range(2):
                    for hh in range(2):
                        nc.vector.tensor_copy(
                            qT_sb[p][:, tcx * 1024 + hh * 512:
                                     tcx * 1024 + (hh + 1) * 512],
                            pq[64 * hh:64 * hh + 64, p * 512:(p + 1) * 512])
                nc.vector.tensor_copy(kT_sb[:, cols], pkv[0:64, :])
                vt = vt_pool.tile([66, 512], BF16, name="vt")
                nc.vector.tensor_copy(vt[0:64, :], pkv[64:128, :])
                nc.sync.dma_start(vt[64:66, :], vones[:, cols])
                for j in range(d.TOK_CHUNK // 128):
                    blk = tcx * (d.TOK_CHUNK // 128) + j
                    ptv = tp_psum.tile([128, 128], BF16, name="ptv")
                    nc.tensor.transpose(
                        ptv[0:128, 0:66],
                        vt[0:66, j * 128:(j + 1) * 128], id66_sb[:])
                    nc.vector.tensor_copy(
                        v_sb[:, blk * 66:(blk + 1) * 66], ptv[0:128, 0:66])

            def emit_scores_kb(u, kb, expT, big_psum):
                b, tcx = u["b"], u["tcx"]
                kcols = slice(b * d.S + kb * 128, b * d.S + (kb + 1) * 128)
                for p in range(2):
                    st = big_psum.tile([128, 1024], F32, tag="big", name="st")
                    for hh in range(2):
                        qcols = slice(tcx * 1024 + hh * 512,
                                      tcx * 1024 + (hh + 1) * 512)
                        nc.tensor.matmul(
                            st[:, 512 * hh:512 * hh + 512],
                            kT_sb[:, kcols], qT_sb[p][:, qcols],
                            start=True, stop=True)
                    et = exp_pool.tile([128, 1024], BF16, name="et")
                    nc.scalar.activation(
                        et[:], st[:], mybir.ActivationFunctionType.Exp)
                    expT[kb, p] = et

            def emit_pv_kb(u, kb, expT):
                b = u["b"]
                vblk = (b * d.S) // 128 + kb
                for p in range(2):
                    for hh in range(2):
                        nc.tensor.matmul(
                            u["attnT_ps"][p][:, 512 * hh:512 * hh + 512],
                            v_sb[:, vblk * 66:vblk * 66 + 65],
                            expT[kb, p][:, 512 * hh:512 * hh + 512],
                            start=(kb == 0), stop=(kb == d.N_KB - 1))

            def emit_dn(u):
                # denominators -> dn rows 0/32/64/96 (head 2p+hh -> 32*(2p+hh))
                attnT_ps = u["attnT_ps"]
                for p in range(2):
                    for hh in range(2):
                        nc.vector.tensor_copy(
                            dn_sb[32 * (2 * p + hh):32 * (2 * p + hh) + 1, :],
                            attnT_ps[p][64:65, 512 * hh:512 * hh + 512])

            def emit_norm(u, big_psum):
                attnT_ps = u["attnT_ps"]
                bc_ps = big_psum.tile([128, 1024], F32, tag="big",
                                      name="bc_ps")
                bc_sb = norm_pool.tile([128, 1024], F32, name="bc_sb")
                for p in range(2):
                    nc.tensor.matmul(
                        bc_ps[:, 512 * p:512 * p + 512],
                        emat_sb[:, 128 * p:128 * (p + 1)],
                        dn_sb[:], start=True, stop=True)
                nc.vector.reciprocal_approx_fast(out=bc_sb[:], in_=bc_ps[:])
                for p in range(2):
                    for hh in range(2):
                        nc.vector.tensor_mul(
                            attnT_sb[64 * hh:64 * hh + 64,
                                     p * d.TOK_CHUNK:(p + 1) * d.TOK_CHUNK],
                            attnT_ps[p][0:64, 512 * hh:512 * hh + 512],
                            bc_sb[64 * hh:64 * hh + 64,
                                  512 * p:512 * p + 512])

            def emit_o(u, big_psum, qs_list):
                b, qc = u["b"], u["qc"]
                for qs in qs_list:
                    rows = slice(b * d.S + qc * d.TOK_CHUNK + qs * 128,
                                 b * d.S + qc * d.TOK_CHUNK + (qs + 1) * 128)
                    for nh in range(NNH):
                        po = big_psum.tile([128, 1024], F32, tag="big",
                                           name="po")
                        for ct in range(2):
                            for nn in range(2):
                                nc.tensor.matmul(
                                    po[:, nn * 512:(nn + 1) * 512],
                                    attnT_sb[:, ct * d.TOK_CHUNK + qs * 128:
                                             ct * d.TOK_CHUNK
                                             + (qs + 1) * 128],
                                    wo_sb[:, ct * d.OUT + nh * 1024
                                          + nn * 512:
                                          ct * d.OUT + nh * 1024
                                          + (nn + 1) * 512],
                                    start=(ct == 0), stop=(ct == 1))
                        ot = ostage_pool.tile([128, 1024], BF16, name="ot")
                        nc.vector.tensor_copy(ot[:], po[:])
                        nc.sync.dma_start(
                            out[rows, nh * 1024:(nh + 1) * 1024], ot[:])

            for _rep in range(repeat):
                units = []
                for b_ in range(d.B):
                    for qc in range(d.N_QC):
                        units.append({
                            "b": b_, "qc": qc,
                            "tcx": b_ * d.N_QC + qc,
                        })
                # overlap unit 0's scores/exp (ScalarE work) into the back
                # half of the projection phase; needs batch-0 projections
                # (chunks 0..N_TC/2-1) done and a 4-bank proj pool config
                overlap = d.N_TC == 8 and d.NSB >= 8
                half = d.N_TC // 2
                expT0 = {}
                PRE = 6 if overlap else 0

                # ---- projection phase A (batch 0, full pools) ----------
                with (
                    tc.tile_pool(name="pqp", bufs=2, space="PSUM") as pq_pool,
                    tc.tile_pool(name="kvp", bufs=2, space="PSUM") as kv_pool,
                    tc.tile_pool(name="tpp", bufs=2, space="PSUM") as tp_psum,
                ):
                    for tcx in range(half if overlap else d.N_TC):
                        emit_proj(tcx, pq_pool, kv_pool, tp_psum)
                        if tcx == 0 and _rep == 0:
                            nc.sync.dma_start(
                                wo_sb[:, 0:d.OUT], wo[0, :, :])
                            nc.sync.dma_start(
                                wo_sb[:, d.OUT:2 * d.OUT], wo[1, :, :])

                with tc.tile_pool(name="big", bufs=2,
                                  space="PSUM") as big_psum:
                    if overlap:
                        # ---- phase B: batch-1 projections (narrow pools)
                        # interleaved with unit 0's first PRE score blocks
                        kb_per_chunk = [(0, 1), (2,), (3, 4), (5,)]
                        with (
                            tc.tile_pool(name="pqp2", bufs=1,
                                         space="PSUM") as pq2_pool,
                            tc.tile_pool(name="kvp2", bufs=1,
                                         space="PSUM") as kv2_pool,
                            tc.tile_pool(name="tpp2", bufs=1,
                                         space="PSUM") as tp2_psum,
                        ):
                            for ci, tcx in enumerate(range(half, d.N_TC)):
                                emit_proj(tcx, pq2_pool, kv2_pool, tp2_psum)
                                for kb in kb_per_chunk[ci]:
                                    emit_scores_kb(units[0], kb, expT0,
                                                   big_psum)

                    # ---- attention phase --------------------------------
                    with tc.tile_pool(name="attn", bufs=2,
                                      space="PSUM") as attn_ps:
                        if d.NSB >= 6:
                            o_sbs = {3: [0], 4: [1], 5: [2], 6: [3]}
                        else:
                            o_sbs = {1: list(range(NQS))}
                        prev = None
                        for u in units:
                            pre = PRE if u is units[0] else 0
                            expT = expT0 if u is units[0] else {}
                            LAG = 4 if d.NSB >= 4 else 2
                            start_sb = pre // 2
                            for sb in range(start_sb, d.NSB):
                                for kb in range(2 * sb, 2 * sb + 2):
                                    if kb >= pre:
                                        emit_scores_kb(u, kb, expT, big_psum)
                                    if kb >= LAG:
                                        if u.get("attnT_ps") is None:
                                            u["attnT_ps"] = [
                                                attn_ps.tile(
                                                    [65, 1024], F32,
                                                    tag="at",
                                                    name=f"attnT_ps{p}")
                                                for p in range(2)]
                                            for ckb in range(kb - LAG):
                                                emit_pv_kb(u, ckb, expT)
                                        emit_pv_kb(u, kb - LAG, expT)
                                if sb == start_sb and prev is not None:
                                    emit_norm(prev, big_psum)
                                if prev is not None and sb in o_sbs:
                                    emit_o(prev, big_psum, o_sbs[sb])
                            for kb in range(d.N_KB - LAG, d.N_KB):
                                emit_pv_kb(u, kb, expT)
                            emit_dn(u)
                            prev = u
                        emit_norm(prev, big_psum)
                        emit_o(prev, big_psum, list(range(NQS)))

    nc.compile()
    return nc


def _rope_fold(W, cos, sin, nheads, scale):
    """Fold head-constant RoPE (and scale) into a projection weight.
    W: [HID, nheads*64] fp32; cos/sin: [nheads, 64]."""
    W4 = W.reshape(W.shape[0], nheads, 64)
    out = np.empty_like(W4)
    out[:, :, :32] = W4[:, :, :32] * cos[None, :, :32] \
        - W4[:, :, 32:] * sin[None, :, :32]
    out[:, :, 32:] = W4[:, :, 32:] * cos[None, :, 32:] \
        + W4[:, :, :32] * sin[None, :, 32:]
    return (out * scale).reshape(W.shape)


def _pack_w16(W):
    """W [HID, M] fp32 -> ktile-major [128, KT*M] bf16, where virtual
    ktile 2P+j holds rows {P*256 + 2p + j} (the row order produced by
    DMA-flattening a [256, tok] h slice into a [128, 1024] tile)."""
    HIDd, M = W.shape
    Wp = W.reshape(HIDd // 256, 128, 2, M).transpose(2, 0, 1, 3) \
          .reshape(2, HIDd // 256, 128, M)
    # virtual tile (P, j) = Wp[j, P]; order tiles as 2P+j
    tiles = [Wp[j, P] for P in range(HIDd // 256) for j in range(2)]
    return np.ascontiguousarray(
        np.stack(tiles, 0).transpose(1, 0, 2).reshape(128, -1).astype(bf16))


_PROGRAM_CACHE = {}


def _get_program():
    if "nc" not in _PROGRAM_CACHE:
        _PROGRAM_CACHE["nc"] = build_program(Dims())
    return _PROGRAM_CACHE["nc"]


def make_in_maps(hidden_states, Wq, Wk, Wv, Wo, cos, sin, d: Dims = None):
    """Host-side sharding/prep. Returns per-core input dicts."""
    d = d or Dims()
    hs = np.asarray(hidden_states, np.float32).reshape(d.BS, d.HID)
    hT = np.ascontiguousarray(hs.T.astype(bf16))
    cos = np.asarray(cos, np.float32)
    sin = np.asarray(sin, np.float32)
    nq_total = N_CORES * d.NQ
    Wq_f = _rope_fold(np.asarray(Wq, np.float32), cos[:nq_total],
                      sin[:nq_total], nq_total, 1.0 / np.sqrt(d.HD))
    Wk_f = _rope_fold(np.asarray(Wk, np.float32), cos[:KVH], sin[:KVH],
                      KVH, 1.0)
    Wv_f = np.asarray(Wv, np.float32)
    Wo_f = np.asarray(Wo, np.float32)
    emat = np.zeros([128, 256], np.float32)
    for h in range(4):
        p, hh = h // 2, h % 2
        emat[32 * h, 128 * p + 64 * hh:128 * p + 64 * hh + 64] = 1.0
    id66 = np.eye(66, dtype=bf16)
    vones = np.concatenate([np.ones([1, d.BS], bf16),
                            np.zeros([1, d.BS], bf16)])
    rzero = np.zeros([128, 512], np.float32)
    in_maps = []
    for c in range(N_CORES):
        wq_c = _pack_w16(Wq_f[:, c * d.QCH:(c + 1) * d.QCH])
        wkv_c = _pack_w16(np.concatenate(
            [Wk_f[:, c * d.HD:(c + 1) * d.HD],
             Wv_f[:, c * d.HD:(c + 1) * d.HD]], axis=1))
        wo_c = np.ascontiguousarray(
            Wo_f[c * d.QCH:(c + 1) * d.QCH, :].reshape(2, 128, d.OUT)
            .astype(bf16))
        in_maps.append({
            "h16": hT, "wq16": wq_c, "wkv16": wkv_c,
            "wo": wo_c, "emat": emat, "id66": id66, "vones": vones,
            "rzero": rzero,
        })
    return in_maps


def kernel(hidden_states, Wq, Wk, Wv, Wo, cos, sin):
    d = Dims()
    nc = _get_program()
    in_maps = make_in_maps(hidden_states, Wq, Wk, Wv, Wo, cos, sin, d)
    res = bass_utils.run_bass_kernel_spmd(
        nc, in_maps, core_ids=list(range(N_CORES)))
    acc = res.results[0]["out"].astype(np.float32)
    for c in range(1, N_CORES):
        acc += res.results[c]["out"].astype(np.float32)
    return acc.reshape(B, S, HID)


if __name__ == "__main__":
    rng = np.random.default_rng(0)
    h = rng.standard_normal((B, S, HID), dtype=np.float32)
    sc = 1.0 / np.sqrt(HID)
    Wq_ = rng.standard_normal((HID, NH * HD), dtype=np.float32) * sc
    Wk_ = rng.standard_normal((HID, KVH * HD), dtype=np.float32) * sc
    Wv_ = rng.standard_normal((HID, KVH * HD), dtype=np.float32) * sc
    Wo_ = rng.standard_normal((NH * HD, HID), dtype=np.float32) * sc
    inv = 1.0 / (10000.0 ** (np.arange(0, HD, 2, dtype=np.float32) / HD))
    t = np.arange(S, dtype=np.float32)
    fr = np.outer(t, inv)
    emb = np.concatenate([fr, fr], axis=-1)
    o = kernel(h, Wq_, Wk_, Wv_, Wo_, np.cos(emb), np.sin(emb))
    print("out", o.shape, o.dtype, float(np.abs(o).max()))


# revision 31
# speedup vs baseline: 1.9543x; 1.0062x over previous
"""Trainium2 Bass kernel for nn_AvaAttention (GQA attention, head-constant RoPE).

Sharding: tensor-parallel over the 8 kv heads -> core c owns kv head c and
q heads 4c..4c+3. Each core computes its 4 heads' attention and a partial
o_proj (row-split Wo); the host sums the 8 partials.

v2 design (per-core):
- RoPE is head-constant (the module indexes cos/sin by head), so it folds
  into Wq/Wk on the host along with the 1/sqrt(D) scale.
- The whole pipeline runs in bf16 (hidden states DMA'd as bf16 -> half
  the input traffic of the fp32 baseline; q/k/v, exp probs, attn and Wo
  all bf16). Measured end-to-end error ~7e-3, well under the 2e-2 gate;
  PE cost for bf16 is the same 1 col/cycle as fp32r.
  (fp8 DoubleRow + hi/lo residual projections were tried and work
  numerically, but each non-fp32 matmul is split into Ldweights+Matmult,
  and at 107ns/MM the extra ~71ns sequencer dispatch makes the PE
  dispatch-bound - no wall-clock win, so bf16 was kept.)
- Scores are computed transposed ([ktok, qtok]) so exp feeds the PV matmul
  directly; a 65th all-ones column of v makes the PV matmul emit the
  softmax denominator (row 64). exp runs on ScalarE ([128,1024] tiles,
  one per (kb, head-pair)).
- q is stored pair-packed ([64, chunk*2*512]: head-even cols then head-odd
  cols per chunk) so scores need no partition-offset staging.
- Softmax normalization is deferred: denominators are broadcast via a
  tiny PE matmul (emat), reciprocaled on DVE, and multiplied into
  attnT_sb (bf16) which feeds o_proj.
- Projections run up front, PE-bound (2 k-tiles per [128,1024] h DMA;
  the SP queue's ~600ns/DMA issue rate made 16-DMA chunks issue-bound).
  Batch-1 projections run with narrowed PSUM pools so unit 0's first 6
  score blocks (and their ScalarE exps) overlap them - ScalarE is
  otherwise idle for the whole projection phase.
- The 8 attention units are software-pipelined: unit u's scores/exp
  overlap unit u-1's PV normalize + o_proj (o_proj token-groups spread
  over sub-blocks 3..6, PV lagging scores by 4 k-blocks).
- PSUM: scores/bcast/po share a 2x[128,1024] ring (4 banks); the two
  per-unit attnT accumulators [65,1024] live in a bufs=2 pool (4 banks);
  projections use their own pools inside contexts that exit before the
  attention pools open.
"""

import numpy as np
import ml_dtypes

import concourse.bass as bass
import concourse.bacc as bacc
import concourse.tile as tile
import concourse.mybir as mybir
from concourse import bass_utils

BF16 = mybir.dt.bfloat16
F32 = mybir.dt.float32
F32R = mybir.dt.float32r
FP8 = mybir.dt.float8e4
DR = mybir.MatmulPerfMode.DoubleRow
bf16 = ml_dtypes.bfloat16
e4m3 = ml_dtypes.float8_e4m3

# Problem dims (hardcoded per contract)
B, S, HID = 2, 2048, 2048
NH, KVH, HD = 32, 8, 64
N_CORES = 8


class Dims:
    """All derived tile counts; parameterized so small variants can be
    simulated in CoreSim."""

    def __init__(self, B=B, S=S, HID=HID, n_qheads=4, HD=HD, out_ch=HID):
        self.B, self.S, self.HID, self.HD = B, S, HID, HD
        self.NQ = n_qheads              # q heads per core (must be 4)
        self.BS = B * S                 # total tokens
        self.QCH = n_qheads * HD        # q channels per core (256)
        self.OUT = out_ch               # o_proj output channels
        self.TOK_CHUNK = 512            # projection/attention token chunk
        self.KT = HID // 128            # contraction tiles for projections
        self.NPAIR = self.KT // 2       # DoubleRow k-tile pairs
        self.N_TC = self.BS // self.TOK_CHUNK
        self.N_QC = S // self.TOK_CHUNK  # q chunks per batch
        self.N_KB = S // 128            # ktok blocks per batch
        self.NSB = self.N_KB // 2
        assert n_qheads == 4 and HD == 64
        assert self.KT % 2 == 0 and self.OUT % 1024 == 0 and S % 512 == 0


def build_program(d: Dims, repeat: int = 1, exp_bufs: int = 22):
    """Emit the per-core SPMD program. Returns compiled nc."""
    nc = bacc.Bacc("TRN2", target_bir_lowering=False, debug=False)

    # ---- DRAM I/O -------------------------------------------------------
    h16 = nc.dram_tensor("h16", [d.HID, d.BS], BF16, kind="ExternalInput")
    # ktile-major packed bf16 weights
    wq16 = nc.dram_tensor("wq16", [128, d.KT * 256], BF16,
                          kind="ExternalInput")
    wkv16 = nc.dram_tensor("wkv16", [128, d.KT * 128], BF16,
                           kind="ExternalInput")
    wo = nc.dram_tensor("wo", [2, 128, d.OUT], BF16, kind="ExternalInput")
    emat = nc.dram_tensor("emat", [128, 256], F32R, kind="ExternalInput")
    id66 = nc.dram_tensor("id66", [66, 66], BF16, kind="ExternalInput")
    vones = nc.dram_tensor("vones", [2, d.BS], BF16, kind="ExternalInput")
    rzero = nc.dram_tensor("rzero", [128, 512], F32R, kind="ExternalInput")
    out = nc.dram_tensor("out", [d.BS, d.OUT], BF16, kind="ExternalOutput")

    NQS = d.TOK_CHUNK // 128            # 4 o_proj token groups per unit
    NNH = d.OUT // 1024                 # o_proj 1024-wide col blocks

    with tile.TileContext(nc) as tc:
        with (
            tc.tile_pool(name="consts", bufs=1) as consts,
            tc.tile_pool(name="persist", bufs=1) as persist,
            tc.tile_pool(name="ht", bufs=8) as ht_pool,
            tc.tile_pool(name="expp", bufs=exp_bufs) as exp_pool,
            tc.tile_pool(name="norm", bufs=2) as norm_pool,
            tc.tile_pool(name="ostage", bufs=6) as ostage_pool,
            tc.tile_pool(name="vt", bufs=2) as vt_pool,
        ):
            # ---- constants/weights in SBUF ------------------------------
            wq_sb = consts.tile([128, d.KT * 256], BF16, tag="wq")
            wkv_sb = consts.tile([128, d.KT * 128], BF16, tag="wkv")
            wo_sb = consts.tile([128, 2 * d.OUT], BF16, tag="wo")
            emat_sb = consts.tile([128, 256], F32R, tag="emat")
            id66_sb = consts.tile([66, 66], BF16, tag="id66")
            # stage weight DMAs so the first proj matmul waits only the
            # first quarter of wq; the rest land between P-iterations of
            # chunk 0 (emission-ordered ahead of their first reader)
            if d.KT == 16:
                wq_q = [(wq_sb[:, i * 1024:(i + 1) * 1024],
                         wq16[:, i * 1024:(i + 1) * 1024]) for i in range(4)]
                wkv_h = [(wkv_sb[:, i * 1024:(i + 1) * 1024],
                          wkv16[:, i * 1024:(i + 1) * 1024])
                         for i in range(2)]
                nc.sync.dma_start(*wq_q[0])
                nc.sync.dma_start(*wkv_h[0])
                wpieces = [wq_q[1], wkv_h[1], wq_q[2], wq_q[3]]
            else:
                nc.sync.dma_start(wq_sb[:], wq16[:])
                nc.sync.dma_start(wkv_sb[:], wkv16[:])
                wpieces = []
            nc.sync.dma_start(emat_sb[:], emat[:])
            nc.sync.dma_start(id66_sb[:], id66[:])

            # ---- persistent activations --------------------------------
            # q pair-packed: [64, chunk*1024]; chunk c cols =
            # [head-even 512 | head-odd 512]
            qT_sb = [persist.tile([64, d.N_TC * 1024], BF16, tag=f"qT{p}",
                                  name=f"qT{p}") for p in range(2)]
            kT_sb = persist.tile([64, d.BS], BF16, tag="kT")
            v_sb = persist.tile([128, (d.BS // 128) * 66], BF16, tag="v")
            attnT_sb = persist.tile([128, 2 * d.TOK_CHUNK], BF16,
                                    tag="attnT")
            dn_sb = persist.tile([128, 512], F32R, tag="dn")
            nc.sync.dma_start(dn_sb[:], rzero[:])

            def emit_proj(tcx, pq_pool, kv_pool, tp_psum, sprinkle=None):
                cols = slice(tcx * d.TOK_CHUNK, (tcx + 1) * d.TOK_CHUNK)
                pq = pq_pool.tile([128, 1024], F32, tag="pq", name="pq")
                pkv = kv_pool.tile([128, 512], F32, tag="pkv", name="pkv")
                # one [128,1024] piece per 256-row block: col-half j holds
                # the block's rows {2p+j} (row-major DMA flatten), matching
                # the host's even/odd-interleaved W packing
                for P in range(d.KT // 2):
                    if sprinkle and P in sprinkle:
                        sprinkle[P]()
                    if tcx == 0 and 1 <= P <= len(wpieces):
                        nc.sync.dma_start(*wpieces[P - 1])
                    htt = ht_pool.tile([128, 1024], BF16, name="htt")
                    nc.sync.dma_start(
                        htt[:], h16[P * 256:(P + 1) * 256, cols])
                    for j in range(2):
                        kt = 2 * P + j
                        fl = dict(start=(kt == 0), stop=(kt == d.KT - 1))
                        for m in range(2):
                            nc.tensor.matmul(
                                pq[:, m * 512:(m + 1) * 512],
                                wq_sb[:, kt * 256 + m * 128:
                                      kt * 256 + (m + 1) * 128],
                                htt[:, j * 512:(j + 1) * 512], **fl)
                        nc.tensor.matmul(
                            pkv[:], wkv_sb[:, kt * 128:(kt + 1) * 128],
                            htt[:, j * 512:(j + 1) * 512], **fl)
                # copies out (bf16)
                for p in range(2):
                    for hh in range(2):
                        nc.vector.tensor_copy(
                            qT_sb[p][:, tcx * 1024 + hh * 512:
                                     tcx * 1024 + (hh + 1) * 512],
                            pq[64 * hh:64 * hh + 64, p * 512:(p + 1) * 512])
                nc.vector.tensor_copy(kT_sb[:, cols], pkv[0:64, :])
                vt = vt_pool.tile([66, 512], BF16, name="vt")
                nc.vector.tensor_copy(vt[0:64, :], pkv[64:128, :])
                nc.sync.dma_start(vt[64:66, :], vones[:, cols])
                for j in range(d.TOK_CHUNK // 128):
                    blk = tcx * (d.TOK_CHUNK // 128) + j
                    ptv = tp_psum.tile([128, 128], BF16, name="ptv")
                    nc.tensor.transpose(
                        ptv[0:128, 0:66],
                        vt[0:66, j * 128:(j + 1) * 128], id66_sb[:])
                    nc.vector.tensor_copy(
                        v_sb[:, blk * 66:(blk + 1) * 66], ptv[0:128, 0:66])

            def emit_scores_kbp(u, kb, p, expT, psum_pool):
                b, tcx = u["b"], u["tcx"]
                kcols = slice(b * d.S + kb * 128, b * d.S + (kb + 1) * 128)
                st = psum_pool.tile([128, 1024], F32, tag="big", name="st")
                for hh in range(2):
                    qcols = slice(tcx * 1024 + hh * 512,
                                  tcx * 1024 + (hh + 1) * 512)
                    nc.tensor.matmul(
                        st[:, 512 * hh:512 * hh + 512],
                        kT_sb[:, kcols], qT_sb[p][:, qcols],
                        start=True, stop=True)
                et = exp_pool.tile([128, 1024], BF16, name="et")
                nc.scalar.activation(
                    et[:], st[:], mybir.ActivationFunctionType.Exp)
                expT[kb, p] = et

            def emit_scores_kb(u, kb, expT, big_psum):
                for p in range(2):
                    emit_scores_kbp(u, kb, p, expT, big_psum)

            def emit_pv_kb(u, kb, expT):
                b = u["b"]
                vblk = (b * d.S) // 128 + kb
                for p in range(2):
                    for hh in range(2):
                        nc.tensor.matmul(
                            u["attnT_ps"][p][:, 512 * hh:512 * hh + 512],
                            v_sb[:, vblk * 66:vblk * 66 + 65],
                            expT[kb, p][:, 512 * hh:512 * hh + 512],
                            start=(kb == 0), stop=(kb == d.N_KB - 1))

            def emit_dn(u):
                # denominators -> dn rows 0/32/64/96 (head 2p+hh -> 32*(2p+hh))
                attnT_ps = u["attnT_ps"]
                for p in range(2):
                    for hh in range(2):
                        nc.vector.tensor_copy(
                            dn_sb[32 * (2 * p + hh):32 * (2 * p + hh) + 1, :],
                            attnT_ps[p][64:65, 512 * hh:512 * hh + 512])

            def emit_norm(u, big_psum):
                attnT_ps = u["attnT_ps"]
                bc_ps = big_psum.tile([128, 1024], F32, tag="big",
                                      name="bc_ps")
                bc_sb = norm_pool.tile([128, 1024], F32, name="bc_sb")
                for p in range(2):
                    nc.tensor.matmul(
                        bc_ps[:, 512 * p:512 * p + 512],
                        emat_sb[:, 128 * p:128 * (p + 1)],
                        dn_sb[:], start=True, stop=True)
                nc.vector.reciprocal_approx_fast(out=bc_sb[:], in_=bc_ps[:])
                for p in range(2):
                    for hh in range(2):
                        nc.vector.tensor_mul(
                            attnT_sb[64 * hh:64 * hh + 64,
                                     p * d.TOK_CHUNK:(p + 1) * d.TOK_CHUNK],
                            attnT_ps[p][0:64, 512 * hh:512 * hh + 512],
                            bc_sb[64 * hh:64 * hh + 64,
                                  512 * p:512 * p + 512])

            def emit_o(u, big_psum, qs_list):
                b, qc = u["b"], u["qc"]
                for qs in qs_list:
                    rows = slice(b * d.S + qc * d.TOK_CHUNK + qs * 128,
                                 b * d.S + qc * d.TOK_CHUNK + (qs + 1) * 128)
                    for nh in range(NNH):
                        po = big_psum.tile([128, 1024], F32, tag="big",
                                           name="po")
                        for ct in range(2):
                            for nn in range(2):
                                nc.tensor.matmul(
                                    po[:, nn * 512:(nn + 1) * 512],
                                    attnT_sb[:, ct * d.TOK_CHUNK + qs * 128:
                                             ct * d.TOK_CHUNK
                                             + (qs + 1) * 128],
                                    wo_sb[:, ct * d.OUT + nh * 1024
                                          + nn * 512:
                                          ct * d.OUT + nh * 1024
                                          + (nn + 1) * 512],
                                    start=(ct == 0), stop=(ct == 1))
                        ot = ostage_pool.tile([128, 1024], BF16, name="ot")
                        nc.vector.tensor_copy(ot[:], po[:])
                        nc.sync.dma_start(
                            out[rows, nh * 1024:(nh + 1) * 1024], ot[:])

            for _rep in range(repeat):
                units = []
                for b_ in range(d.B):
                    for qc in range(d.N_QC):
                        units.append({
                            "b": b_, "qc": qc,
                            "tcx": b_ * d.N_QC + qc,
                        })
                # overlap unit 0's scores/exp (ScalarE work) into the back
                # half of the projection phase; needs batch-0 projections
                # (chunks 0..N_TC/2-1) done and a 4-bank proj pool config
                overlap = d.N_TC == 8 and d.NSB >= 8
                half = d.N_TC // 2
                expT0 = {}
                PRE = 8 if overlap else 0

                # ---- projection phase A (batch 0, full pools) ----------
                with (
                    tc.tile_pool(name="pqp", bufs=2, space="PSUM") as pq_pool,
                    tc.tile_pool(name="kvp", bufs=2, space="PSUM") as kv_pool,
                    tc.tile_pool(name="tpp", bufs=2, space="PSUM") as tp_psum,
                ):
                    for tcx in range(half if overlap else d.N_TC):
                        emit_proj(tcx, pq_pool, kv_pool, tp_psum)
                        if tcx == 0 and _rep == 0:
                            nc.sync.dma_start(
                                wo_sb[:, 0:d.OUT], wo[0, :, :])
                            nc.sync.dma_start(
                                wo_sb[:, d.OUT:2 * d.OUT], wo[1, :, :])

                if overlap:
                    # ---- phase B: batch-1 projections (full-width pq
                    # pool) with unit 0's first PRE score blocks sprinkled
                    # between P-iterations into a private 1-slot ring
                    with (
                        tc.tile_pool(name="pqp2", bufs=2,
                                     space="PSUM") as pq2_pool,
                        tc.tile_pool(name="kvp2", bufs=1,
                                     space="PSUM") as kv2_pool,
                        tc.tile_pool(name="tpp2", bufs=1,
                                     space="PSUM") as tp2_psum,
                        tc.tile_pool(name="bigB", bufs=1,
                                     space="PSUM") as bigB,
                    ):
                        for ci, tcx in enumerate(range(half, d.N_TC)):
                            def mk(kb, p):
                                return lambda: emit_scores_kbp(
                                    units[0], kb, p, expT0, bigB)
                            spr = {0: mk(2 * ci, 0), 2: mk(2 * ci, 1),
                                   4: mk(2 * ci + 1, 0),
                                   6: mk(2 * ci + 1, 1)}
                            emit_proj(tcx, pq2_pool, kv2_pool, tp2_psum,
                                      sprinkle=spr)

                with (
                    tc.tile_pool(name="big", bufs=2,
                                 space="PSUM") as big_psum,
                    tc.tile_pool(name="attn", bufs=2,
                                 space="PSUM") as attn_ps,
                ):
                    if True:
                        if d.NSB >= 6:
                            o_sbs = {3: [0], 4: [1], 5: [2], 6: [3]}
                        else:
                            o_sbs = {1: list(range(NQS))}
                        prev = None
                        for u in units:
                            pre = PRE if u is units[0] else 0
                            expT = expT0 if u is units[0] else {}
                            LAG = 4 if d.NSB >= 4 else 2
                            start_sb = pre // 2
                            for sb in range(start_sb, d.NSB):
                                for kb in range(2 * sb, 2 * sb + 2):
                                    if kb >= pre:
                                        emit_scores_kb(u, kb, expT, big_psum)
                                    if kb >= LAG:
                                        if u.get("attnT_ps") is None:
                                            u["attnT_ps"] = [
                                                attn_ps.tile(
                                                    [65, 1024], F32,
                                                    tag="at",
                                                    name=f"attnT_ps{p}")
                                                for p in range(2)]
                                            for ckb in range(kb - LAG):
                                                emit_pv_kb(u, ckb, expT)
                                        emit_pv_kb(u, kb - LAG, expT)
                                if sb == start_sb and prev is not None:
                                    emit_norm(prev, big_psum)
                                if prev is not None and sb in o_sbs:
                                    emit_o(prev, big_psum, o_sbs[sb])
                            for kb in range(d.N_KB - LAG, d.N_KB):
                                emit_pv_kb(u, kb, expT)
                            emit_dn(u)
                            prev = u
                        emit_norm(prev, big_psum)
                        emit_o(prev, big_psum, list(range(NQS)))

    nc.compile()
    return nc


def _rope_fold(W, cos, sin, nheads, scale):
    """Fold head-constant RoPE (and scale) into a projection weight.
    W: [HID, nheads*64] fp32; cos/sin: [nheads, 64]."""
    W4 = W.reshape(W.shape[0], nheads, 64)
    out = np.empty_like(W4)
    out[:, :, :32] = W4[:, :, :32] * cos[None, :, :32] \
        - W4[:, :, 32:] * sin[None, :, :32]
    out[:, :, 32:] = W4[:, :, 32:] * cos[None, :, 32:] \
        + W4[:, :, :32] * sin[None, :, 32:]
    return (out * scale).reshape(W.shape)


def _pack_w16(W):
    """W [HID, M] fp32 -> ktile-major [128, KT*M] bf16, where virtual
    ktile 2P+j holds rows {P*256 + 2p + j} (the row order produced by
    DMA-flattening a [256, tok] h slice into a [128, 1024] tile)."""
    HIDd, M = W.shape
    Wp = W.reshape(HIDd // 256, 128, 2, M).transpose(2, 0, 1, 3) \
          .reshape(2, HIDd // 256, 128, M)
    # virtual tile (P, j) = Wp[j, P]; order tiles as 2P+j
    tiles = [Wp[j, P] for P in range(HIDd // 256) for j in range(2)]
    return np.ascontiguousarray(
        np.stack(tiles, 0).transpose(1, 0, 2).reshape(128, -1).astype(bf16))


_PROGRAM_CACHE = {}


def _get_program():
    if "nc" not in _PROGRAM_CACHE:
        _PROGRAM_CACHE["nc"] = build_program(Dims())
    return _PROGRAM_CACHE["nc"]


def make_in_maps(hidden_states, Wq, Wk, Wv, Wo, cos, sin, d: Dims = None):
    """Host-side sharding/prep. Returns per-core input dicts."""
    d = d or Dims()
    hs = np.asarray(hidden_states, np.float32).reshape(d.BS, d.HID)
    hT = np.ascontiguousarray(hs.T.astype(bf16))
    cos = np.asarray(cos, np.float32)
    sin = np.asarray(sin, np.float32)
    nq_total = N_CORES * d.NQ
    Wq_f = _rope_fold(np.asarray(Wq, np.float32), cos[:nq_total],
                      sin[:nq_total], nq_total, 1.0 / np.sqrt(d.HD))
    Wk_f = _rope_fold(np.asarray(Wk, np.float32), cos[:KVH], sin[:KVH],
                      KVH, 1.0)
    Wv_f = np.asarray(Wv, np.float32)
    Wo_f = np.asarray(Wo, np.float32)
    emat = np.zeros([128, 256], np.float32)
    for h in range(4):
        p, hh = h // 2, h % 2
        emat[32 * h, 128 * p + 64 * hh:128 * p + 64 * hh + 64] = 1.0
    id66 = np.eye(66, dtype=bf16)
    vones = np.concatenate([np.ones([1, d.BS], bf16),
                            np.zeros([1, d.BS], bf16)])
    rzero = np.zeros([128, 512], np.float32)
    in_maps = []
    for c in range(N_CORES):
        wq_c = _pack_w16(Wq_f[:, c * d.QCH:(c + 1) * d.QCH])
        wkv_c = _pack_w16(np.concatenate(
            [Wk_f[:, c * d.HD:(c + 1) * d.HD],
             Wv_f[:, c * d.HD:(c + 1) * d.HD]], axis=1))
        wo_c = np.ascontiguousarray(
            Wo_f[c * d.QCH:(c + 1) * d.QCH, :].reshape(2, 128, d.OUT)
            .astype(bf16))
        in_maps.append({
            "h16": hT, "wq16": wq_c, "wkv16": wkv_c,
            "wo": wo_c, "emat": emat, "id66": id66, "vones": vones,
            "rzero": rzero,
        })
    return in_maps


def kernel(hidden_states, Wq, Wk, Wv, Wo, cos, sin):
    d = Dims()
    nc = _get_program()
    in_maps = make_in_maps(hidden_states, Wq, Wk, Wv, Wo, cos, sin, d)
    res = bass_utils.run_bass_kernel_spmd(
        nc, in_maps, core_ids=list(range(N_CORES)))
    acc = res.results[0]["out"].astype(np.float32)
    for c in range(1, N_CORES):
        acc += res.results[c]["out"].astype(np.float32)
    return acc.reshape(B, S, HID)


if __name__ == "__main__":
    rng = np.random.default_rng(0)
    h = rng.standard_normal((B, S, HID), dtype=np.float32)
    sc = 1.0 / np.sqrt(HID)
    Wq_ = rng.standard_normal((HID, NH * HD), dtype=np.float32) * sc
    Wk_ = rng.standard_normal((HID, KVH * HD), dtype=np.float32) * sc
    Wv_ = rng.standard_normal((HID, KVH * HD), dtype=np.float32) * sc
    Wo_ = rng.standard_normal((NH * HD, HID), dtype=np.float32) * sc
    inv = 1.0 / (10000.0 ** (np.arange(0, HD, 2, dtype=np.float32) / HD))
    t = np.arange(S, dtype=np.float32)
    fr = np.outer(t, inv)
    emb = np.concatenate([fr, fr], axis=-1)
    o = kernel(h, Wq_, Wk_, Wv_, Wo_, np.cos(emb), np.sin(emb))
    print("out", o.shape, o.dtype, float(np.abs(o).max()))
